# revision 1
# baseline (speedup 1.0000x reference)
"""ALiBi multi-head causal attention on 8 TRN2 NeuronCores.

Sharding: core = b*4 + hg  (b in 0..1 batches, hg in 0..3 head-groups).
Each core computes 4 heads of one batch end-to-end (KQV projection for its
head-columns + causal ALiBi attention).  No collectives needed.

Per-core kernel (all matmuls bf16, f32 accumulation):
  - xT   [D, S]   : x[b].T -- contraction dim D on partitions
  - kqT  = (x W_kq)^T computed as [hd, S] per head (head-dim on partitions)
  - v    = x W_v computed as [S, hd] row-blocks (1/sqrt(hd) folded into q)
  - attention runs per head in groups of 4 query-blocks (512 columns) and
    works entirely in TRANSPOSED score space scoreT[t, sq] (k stationary,
    q-group moving), so the PV matmul consumes probsT directly and no
    per-chunk transposes exist anywhere:
      probsT[t,sq] = exp(scoreT + m*(tl-sqg) [+ causal -1e30] + shift(c-4G))
    No softmax max-subtraction (exponent <= score <= O(10) since the alibi
    bias is <= 0 in the causal region); far-past underflow to 0 is exact.
  - rowsum over t (partition axis) via M=1 ones-matmuls accumulated in PSUM;
    1/rowsum broadcast across partitions via a K=1 f32r matmul and
    reciprocal_approx_fast; out = (probsT-PV) * bcast(1/rowsum) + b_v.
  - output written as outT [head, hd, s]; host transposes back.
"""

import sys

if "/opt/trn_rl_repo" not in sys.path:
    sys.path.insert(0, "/opt/trn_rl_repo")

import numpy as np
import ml_dtypes

import concourse.bass as bass
import concourse.mybir as mybir
from concourse import bacc
from concourse.tile import TileContext
from concourse.bass_utils import run_bass_kernel_spmd

P = 128
S = 2048
D = 2048
HD = 128
NB = S // P            # 16 seq blocks
H_LOC = 4              # heads per core
NUM_HEADS = 16
SCALE = 1.0 / np.sqrt(HD)

F32 = mybir.dt.float32
F32R = mybir.dt.float32r
BF16 = mybir.dt.bfloat16
AF = mybir.ActivationFunctionType
OP = mybir.AluOpType


def _alibi_slopes(num_heads=NUM_HEADS):
    base = (2.0 ** 8) ** (1.0 / num_heads)
    return np.asarray([1.0 / base ** (i + 1) for i in range(num_heads)], np.float32)


def build():
    nc = bacc.Bacc("TRN2", target_bir_lowering=False)

    xT_d = nc.declare_dram_parameter("xT", [D, S], BF16, isOutput=False)
    wKQ_d = nc.declare_dram_parameter("wKQ", [D, 8 * P], BF16, isOutput=False)
    wV_d = nc.declare_dram_parameter("wV", [D, H_LOC * HD], BF16, isOutput=False)
    bKQ_d = nc.declare_dram_parameter("bKQ", [P, 8], F32, isOutput=False)
    bVT_d = nc.declare_dram_parameter("bVT", [HD, H_LOC], F32, isOutput=False)
    # transposed-space bias merged with causal mask variants:
    # biasT[j, 0] = m_j*(tl-sqg); biasT[j, 1+d] additionally has -1e30 where
    # tl > sql inside diagonal block d (d = chunk - 4G in 0..3)
    biasT_d = nc.declare_dram_parameter(
        "biasT", [H_LOC, 5, P, 512], F32, isOutput=False
    )
    # per-chunk shift: negshT[p, j, d+12] = m_j * 128 * d   (d = c - 4G)
    negshT_d = nc.declare_dram_parameter("negshT", [P, H_LOC, 16], F32, isOutput=False)
    # out in transposed-per-head layout [head, hd, s]; host transposes back
    out_d = nc.declare_dram_parameter("out", [H_LOC, HD, S], F32, isOutput=True)

    xT_t = xT_d.rearrange("(ko p) s -> p ko s", p=P)     # [128, 16, 2048]
    wKQ_t = wKQ_d.rearrange("(ko p) n -> p ko n", p=P)   # [128, 16, 1024]
    wV_t = wV_d.rearrange("(ko p) n -> p ko n", p=P)     # [128, 16, 512]

    with TileContext(nc) as tc:
        with (
            tc.tile_pool(name="const", bufs=1) as const,
            tc.tile_pool(name="resid", bufs=1) as resid,
            tc.tile_pool(name="stats", bufs=4) as stats,
            tc.tile_pool(name="psA", bufs=3, space="PSUM") as psA,
            tc.tile_pool(name="psO", bufs=2, space="PSUM") as psO,
            tc.tile_pool(name="psS", bufs=2, space="PSUM") as psS,
            tc.tile_pool(name="wpool", bufs=1) as wpool,
            tc.tile_pool(name="xpool", bufs=2) as xpool,
            tc.tile_pool(name="attn", bufs=2) as attn_pool,
            tc.tile_pool(name="biasp", bufs=2) as bias_pool,
        ):
            # ---- constants ----
            bkq_sb = const.tile([P, 8], F32)
            nc.sync.dma_start(bkq_sb, bKQ_d[:])
            bvt_sb = const.tile([HD, H_LOC], F32)
            nc.sync.dma_start(bvt_sb, bVT_d[:])

            negshT = const.tile([P, H_LOC, 16], F32)
            nc.sync.dma_start(negshT, negshT_d[:])

            ones_bf = const.tile([P, 1], BF16)  # rowsum column
            nc.gpsimd.memset(ones_bf, 1.0)
            ones1_raw = const.tile([1, P], F32)
            nc.gpsimd.memset(ones1_raw, 1.0)
            ones1_f = const.tile([1, P], F32R)  # partition-broadcast row
            with nc.allow_low_precision(reason="constant ones cast to f32r"):
                nc.vector.tensor_copy(ones1_f, ones1_raw)

            # ---- residents ----
            kq_all = resid.tile([P, 8, S], BF16)       # [hd, (K h0..3 | Q h0..3), s]
            v_all = resid.tile([P, NB, H_LOC * HD], BF16)  # [si, so, j*128+d]

            # ---- phase 1: KQV projection ----
            wkq_sb = wpool.tile([P, 16, 8 * P], BF16)
            for m in range(8):
                nc.sync.dma_start(
                    wkq_sb[:, :, m * P : (m + 1) * P],
                    wKQ_t[:, :, m * P : (m + 1) * P],
                )
            wv_sb = wpool.tile([P, 16, H_LOC * HD], BF16)
            nc.sync.dma_start(wv_sb, wV_t)

            for nb in range(S // 512):
                xc = xpool.tile([P, 16, 512], BF16, tag="xc")
                for kk in range(4):
                    nc.scalar.dma_start(
                        xc[:, 4 * kk : 4 * kk + 4, :],
                        xT_t[:, 4 * kk : 4 * kk + 4, nb * 512 : (nb + 1) * 512],
                    )
                for m in range(8):
                    ps = psA.tile([P, 512], F32, tag="ps")
                    for k in range(16):
                        nc.tensor.matmul(
                            ps,
                            lhsT=wkq_sb[:, k, m * P : (m + 1) * P],
                            rhs=xc[:, k, :],
                            start=(k == 0),
                            stop=(k == 15),
                        )
                    # kqT = psum * scale + bias (scale folds 1/sqrt(hd) into q)
                    nc.scalar.activation(
                        kq_all[:, m, nb * 512 : (nb + 1) * 512],
                        ps,
                        AF.Identity,
                        bias=bkq_sb[:, m : m + 1],
                        scale=float(SCALE) if m >= 4 else 1.0,
                    )
                for sub in range(4):
                    s_idx = nb * 4 + sub
                    psv = psA.tile([P, 512], F32, tag="ps")
                    for k in range(16):
                        nc.tensor.matmul(
                            psv,
                            lhsT=xc[:, k, sub * P : (sub + 1) * P],
                            rhs=wv_sb[:, k, :],
                            start=(k == 0),
                            stop=(k == 15),
                        )
                    nc.vector.tensor_copy(v_all[:, s_idx, :], psv)

            # ---- phase 2: attention, transposed score space ----
            # scoreT[t, sq]: k stationary, q-group moving (N=512).  Softmax
            # needs only elementwise ops (bias/mask/exp) + a partition-axis
            # rowsum (M=1 ones-matmul).  PV consumes probsT directly -- no
            # per-chunk transposes anywhere.  Only the causally-valid column
            # range [lo:512] of each chunk is computed; the rest is zeroed.
            for j in range(H_LOC):
                biasT = bias_pool.tile([P, 5, 512], F32, tag="biasT")
                nc.sync.dma_start(biasT, biasT_d[j].rearrange("v p s -> p v s"))
                for G in range(NB // 4):
                    last_c = 4 * G + 3
                    # probsT[t, c, group_col]
                    probsT = attn_pool.tile([P, NB, 512], BF16, tag="pT")
                    rs_ps = psS.tile([1, 512], F32, tag="rs")
                    for c in range(last_c + 1):
                        d = c - 4 * G  # -12..3
                        lo = max(0, d) * P  # first causally-valid column
                        if lo > 0:
                            nc.vector.memset(probsT[:, c, :lo], 0.0)
                        w = 512 - lo
                        ps = psA.tile([P, 512], F32, tag="ps")
                        nc.tensor.matmul(
                            ps[:, :w],
                            lhsT=kq_all[:, j, c * P : (c + 1) * P],
                            rhs=kq_all[:, 4 + j, G * 512 + lo : (G + 1) * 512],
                            start=True,
                            stop=True,
                        )
                        v_idx = 1 + d if d >= 0 else 0
                        scoreT = attn_pool.tile([P, 512], F32, tag="scT")
                        nc.vector.tensor_tensor(
                            scoreT[:, lo:], ps[:, :w], biasT[:, v_idx, lo:], OP.add
                        )
                        nc.scalar.activation(
                            probsT[:, c, lo:],
                            scoreT[:, lo:],
                            AF.Exp,
                            bias=negshT[:, j, d + 12 : d + 13],
                            scale=1.0,
                        )
                        nc.tensor.matmul(
                            rs_ps,
                            lhsT=ones_bf,
                            rhs=probsT[:, c, :],
                            start=(c == 0),
                            stop=(c == last_c),
                        )
                    # PV: outT[hd, sq_group] accumulated over t-chunks
                    po = psO.tile([P, 512], F32, tag="po")
                    for c in range(last_c + 1):
                        nc.tensor.matmul(
                            po,
                            lhsT=v_all[:, c, j * HD : (j + 1) * HD],
                            rhs=probsT[:, c, :],
                            start=(c == 0),
                            stop=(c == last_c),
                        )
                    # normalize: reciprocal of rowsum, broadcast across
                    # partitions on GpSimd, then one DVE multiply
                    rs_sb = stats.tile([1, 512], F32R, tag="rs_sb")
                    with nc.allow_low_precision(reason="f32r rounding only"):
                        nc.vector.tensor_copy(rs_sb, rs_ps)
                    rb = psS.tile([P, 512], F32, tag="rb", bufs=1)
                    nc.tensor.matmul(
                        rb, lhsT=ones1_f, rhs=rs_sb, start=True, stop=True
                    )
                    rb_recip = attn_pool.tile([P, 512], F32, tag="rbsb")
                    nc.vector.reciprocal_approx_fast(rb_recip, rb)
                    out_sb = attn_pool.tile([P, 512], F32, tag="osb")
                    nc.vector.tensor_tensor(out_sb, po, rb_recip, OP.mult)
                    # + V-projection bias (sum of normalized probs == 1)
                    nc.scalar.activation(
                        out_sb,
                        out_sb,
                        AF.Identity,
                        bias=bvt_sb[:, j : j + 1],
                        scale=1.0,
                    )
                    nc.sync.dma_start(
                        out_d[j][:, G * 512 : (G + 1) * 512], out_sb
                    )

    nc.finalize()
    return nc


_NC_CACHE = None


def _get_nc():
    global _NC_CACHE
    if _NC_CACHE is None:
        _NC_CACHE = build()
    return _NC_CACHE


def _make_in_maps(x, W_kqv, b_kqv):
    x = np.asarray(x, np.float32)
    W = np.asarray(W_kqv, np.float32)
    b = np.asarray(b_kqv, np.float32)
    slopes = _alibi_slopes()
    in_maps = []
    for core in range(8):
        bi, hg = divmod(core, 4)
        heads = [4 * hg + j for j in range(H_LOC)]
        xT = np.ascontiguousarray(x[bi].T).astype(ml_dtypes.bfloat16)
        wkq = np.concatenate(
            [W[:, h * HD : (h + 1) * HD] for h in heads]
            + [W[:, D + h * HD : D + (h + 1) * HD] for h in heads],
            axis=1,
        ).astype(ml_dtypes.bfloat16)
        wv = np.concatenate(
            [W[:, 2 * D + h * HD : 2 * D + (h + 1) * HD] for h in heads], axis=1
        ).astype(ml_dtypes.bfloat16)
        # bias columns: K h0..h3 then Q h0..h3; q-side prescaled by 1/sqrt(hd)
        bkq = np.stack(
            [b[h * HD : (h + 1) * HD] for h in heads]
            + [b[D + h * HD : D + (h + 1) * HD] * SCALE for h in heads],
            axis=1,
        ).astype(np.float32)
        bvt = np.stack(
            [b[2 * D + h * HD : 2 * D + (h + 1) * HD] for h in heads], axis=1
        ).astype(np.float32)  # [hd, H_LOC]
        # biasT[j, v, tl, sqg]: v=0 plain m_j*(tl-sqg); v=1+d adds -1e30
        # where tl > sql inside diagonal block d
        relT = (np.arange(P)[:, None] - np.arange(512)[None, :]).astype(np.float32)
        base = slopes[heads][:, None, None] * relT[None]  # [4, 128, 512]
        causal_blk = np.where(
            np.arange(P)[:, None] > np.arange(P)[None, :], -1e30, 0.0
        ).astype(np.float32)
        bias_t = np.zeros((H_LOC, 5, P, 512), np.float32)
        bias_t[:, 0] = base
        for dd in range(4):
            v = base.copy()
            v[:, :, dd * P : (dd + 1) * P] += causal_blk[None]
            bias_t[:, 1 + dd] = v
        # negshT[p, j, d+12] = m_j * 128 * d, d in [-12, 3]
        dvals = (np.arange(16) - 12).astype(np.float32) * P
        negsht = np.tile(
            (slopes[heads][:, None] * dvals[None, :])[None], (P, 1, 1)
        ).astype(np.float32)
        in_maps.append(
            dict(
                xT=xT, wKQ=wkq, wV=wv, bKQ=bkq, bVT=bvt,
                biasT=bias_t, negshT=negsht,
            )
        )
    return in_maps


def run(inputs, trace=False, **kw):
    nc = _get_nc()
    in_maps = _make_in_maps(inputs["x"], inputs["W_kqv"], inputs["b_kqv"])
    bkr = run_bass_kernel_spmd(nc, in_maps, core_ids=list(range(8)), trace=trace, **kw)
    B = 2
    out = np.empty((B, NUM_HEADS, S, HD), np.float32)
    for core in range(8):
        bi, hg = divmod(core, 4)
        o = np.asarray(bkr.results[core]["out"])  # [4, 128(hd), 2048(s)]
        for j in range(H_LOC):
            out[bi, 4 * hg + j] = o[j].T
    return out, bkr


def kernel(x, W_kqv, b_kqv):
    out, _ = run({"x": x, "W_kqv": W_kqv, "b_kqv": b_kqv})
    return out



# revision 5
# speedup vs baseline: 1.0822x; 1.0822x over previous
"""ALiBi multi-head causal attention on 8 TRN2 NeuronCores.

Sharding: core = b*4 + hg (b in 0..1 batches, hg in 0..3).  Heads are
INTERLEAVED across cores: core (b, hg) owns heads [hg, 4+hg, 8+hg, 12+hg]
(slot j = head 4j+hg), so every core holds one head from each ALiBi-slope
quartile.  ALiBi decays exponentially per head; far-past key chunks whose
best-case bias is < -20 contribute < 1e-5 relative mass and are skipped
entirely (QK, softmax, PV, rowsum).  Per-slot windows E = chunks kept
beyond the 4 diagonal chunks of each 512-query group; max slope per slot
bounds the error (worst dropped-chunk bias <= -22).  Work per core:
16+25+38+40 = 119 chunk-units vs 160 causal (all cores identical; SPMD).

Per-core kernel (all matmuls bf16, f32 accumulation):
  - phase 1 (KQV projection) identical to causal baseline; DMAs ordered
    critical-first (wkq m=0 k-split + x block 0 ahead of everything) so
    the first matmul issues ~2us in instead of waiting for the full 10 MB.
  - attention in TRANSPOSED score space scoreT[t, sq] per (slot, q-group):
    bias add runs in-place on the QK PSUM bank, exp applies the per-chunk
    shift, probsT bf16.
  - rowsum over t: full-width chunks fold in bf16 quad-trees on the Vector
    engine, then one M=1 ones-matmul per quad (instead of one per chunk);
    diagonal chunks d=1..3 get column-restricted ([128d:512]) rowsum
    matmuls directly, so no memset of non-causal regions anywhere.
  - PV accumulation is column-restricted the same way.
  - normalize: reciprocal on the [1,512] rowsum, broadcast across
    partitions via a K=1 f32r matmul; out = po * bcast(recip) + b_v.
  - output written as outT [slot, hd, s]; host transposes/reorders back.
"""

import sys

if "/opt/trn_rl_repo" not in sys.path:
    sys.path.insert(0, "/opt/trn_rl_repo")

import numpy as np
import ml_dtypes

import concourse.bass as bass
import concourse.mybir as mybir
from concourse import bacc
from concourse.tile import TileContext
from concourse.bass_utils import run_bass_kernel_spmd

P = 128
S = 2048
D = 2048
HD = 128
NB = S // P            # 16 seq blocks
H_LOC = 4              # heads per core
NUM_HEADS = 16
SCALE = 1.0 / np.sqrt(HD)

# chunks kept beyond the diagonal 4, per head-slot (slot j = head 4j+hg).
# Nearest dropped key sits 128E+1 back: worst dropped-key ALiBi bias per
# slot = -m_max*(128E+1): -32.3, -16.1, -12.0, (none dropped).  Verified
# truncation rel-err 1.5e-6 vs full causal on the reference inputs.
WINDOW_E = (1, 2, 6, 16)

F32 = mybir.dt.float32
F32R = mybir.dt.float32r
BF16 = mybir.dt.bfloat16
AF = mybir.ActivationFunctionType
OP = mybir.AluOpType


def _alibi_slopes(num_heads=NUM_HEADS):
    base = (2.0 ** 8) ** (1.0 / num_heads)
    return np.asarray([1.0 / base ** (i + 1) for i in range(num_heads)], np.float32)


def build():
    nc = bacc.Bacc("TRN2", target_bir_lowering=False)

    xT_d = nc.declare_dram_parameter("xT", [D, S], BF16, isOutput=False)
    wKQ_d = nc.declare_dram_parameter("wKQ", [D, 8 * P], BF16, isOutput=False)
    wV_d = nc.declare_dram_parameter("wV", [D, H_LOC * HD], BF16, isOutput=False)
    bKQ_d = nc.declare_dram_parameter("bKQ", [P, 8], F32, isOutput=False)
    bVT_d = nc.declare_dram_parameter("bVT", [HD, H_LOC], F32, isOutput=False)
    # transposed-space bias, compressed: [:, :, 0:512] = base m_j*(tl-sqg);
    # [:, :, 512+128d : 512+128(d+1)] = base's diag block d (cols 128d..)
    # plus -1e30 where tl > sql (causal mask inside the diagonal block)
    biasT_d = nc.declare_dram_parameter(
        "biasT", [H_LOC, P, 1024], F32, isOutput=False
    )
    # per-chunk shift: negshT[p, j, d+12] = m_j * 128 * d   (d = c - 4G)
    negshT_d = nc.declare_dram_parameter("negshT", [P, H_LOC, 16], F32, isOutput=False)
    # out in transposed-per-slot layout [slot, hd, s]; host transposes back
    out_d = nc.declare_dram_parameter("out", [H_LOC, HD, S], F32, isOutput=True)

    xT_t = xT_d.rearrange("(ko p) s -> p ko s", p=P)     # [128, 16, 2048]
    wKQ_t = wKQ_d.rearrange("(ko p) n -> p ko n", p=P)   # [128, 16, 1024]
    wV_t = wV_d.rearrange("(ko p) n -> p ko n", p=P)     # [128, 16, 512]

    with TileContext(nc) as tc:
        with (
            tc.tile_pool(name="const", bufs=1) as const,
            tc.tile_pool(name="resid", bufs=1) as resid,
            tc.tile_pool(name="stats", bufs=2) as stats,
            tc.tile_pool(name="psA", bufs=3, space="PSUM") as psA,
            tc.tile_pool(name="psO", bufs=2, space="PSUM") as psO,
            tc.tile_pool(name="psS", bufs=2, space="PSUM") as psS,
            tc.tile_pool(name="wpool", bufs=1) as wpool,
            tc.tile_pool(name="xpool", bufs=2) as xpool,
            tc.tile_pool(name="attn", bufs=2) as attn_pool,
            tc.tile_pool(name="fold", bufs=2) as fold_pool,
            tc.tile_pool(name="biasp", bufs=2) as bias_pool,
        ):
            # ---- tiles (DMA issue order below is the startup-critical path:
            # wkq m=0 (k-split) + x block 0 go first on their queues) ----
            wkq_sb = wpool.tile([P, 16, 8 * P], BF16)
            wv_sb = wpool.tile([P, 16, H_LOC * HD], BF16)
            bkq_sb = const.tile([P, 8], F32)
            bvt_sb = const.tile([HD, H_LOC], F32)
            negshT = const.tile([P, H_LOC, 16], F32)

            # sync queue: wkq m=0 in k-quarters first, then the rest
            for kk in range(4):
                nc.sync.dma_start(
                    wkq_sb[:, 4 * kk : 4 * kk + 4, 0:P],
                    wKQ_t[:, 4 * kk : 4 * kk + 4, 0:P],
                )
            nc.sync.dma_start(bkq_sb, bKQ_d[:])
            for m in range(1, 8):
                nc.sync.dma_start(
                    wkq_sb[:, :, m * P : (m + 1) * P],
                    wKQ_t[:, :, m * P : (m + 1) * P],
                )
            nc.sync.dma_start(wv_sb, wV_t)
            nc.sync.dma_start(bvt_sb, bVT_d[:])
            nc.sync.dma_start(negshT, negshT_d[:])

            ones_bf = const.tile([P, 1], BF16)  # rowsum column
            nc.gpsimd.memset(ones_bf, 1.0)
            ones1_raw = const.tile([1, P], F32)
            nc.gpsimd.memset(ones1_raw, 1.0)
            ones1_f = const.tile([1, P], F32R)  # partition-broadcast row
            with nc.allow_low_precision(reason="constant ones cast to f32r"):
                nc.vector.tensor_copy(ones1_f, ones1_raw)

            # ---- residents ----
            kq_all = resid.tile([P, 8, S], BF16)       # [hd, (K s0..3 | Q s0..3), s]
            v_all = resid.tile([P, NB, H_LOC * HD], BF16)  # [si, so, j*128+d]

            # ---- phase 1: KQV projection ----
            for nb in range(S // 512):
                xc = xpool.tile([P, 16, 512], BF16, tag="xc")
                for kk in range(4):
                    nc.scalar.dma_start(
                        xc[:, 4 * kk : 4 * kk + 4, :],
                        xT_t[:, 4 * kk : 4 * kk + 4, nb * 512 : (nb + 1) * 512],
                    )
                for m in range(8):
                    ps = psA.tile([P, 512], F32, tag="ps")
                    for k in range(16):
                        nc.tensor.matmul(
                            ps,
                            lhsT=wkq_sb[:, k, m * P : (m + 1) * P],
                            rhs=xc[:, k, :],
                            start=(k == 0),
                            stop=(k == 15),
                        )
                    # kqT = psum * scale + bias (scale folds 1/sqrt(hd) into q)
                    nc.scalar.activation(
                        kq_all[:, m, nb * 512 : (nb + 1) * 512],
                        ps,
                        AF.Identity,
                        bias=bkq_sb[:, m : m + 1],
                        scale=float(SCALE) if m >= 4 else 1.0,
                    )
                for sub in range(4):
                    s_idx = nb * 4 + sub
                    psv = psA.tile([P, 512], F32, tag="ps")
                    for k in range(16):
                        nc.tensor.matmul(
                            psv,
                            lhsT=xc[:, k, sub * P : (sub + 1) * P],
                            rhs=wv_sb[:, k, :],
                            start=(k == 0),
                            stop=(k == 15),
                        )
                    nc.vector.tensor_copy(v_all[:, s_idx, :], psv)

            # ---- phase 2: attention, transposed score space ----
            # Per (slot j, q-group G): kept chunks c in [c_lo, 4G+3] with
            # c_lo = max(0, 4G - E_j).  QK -> in-place bias add on PSUM ->
            # exp(probsT bf16).  Full-width chunks (d <= 0) fold in bf16
            # quad-trees on DVE; one rowsum matmul per quad; diagonal chunks
            # d=1..3 rowsum directly on their causal column range.  PV
            # accumulates column-restricted.  No memsets.
            for j in range(H_LOC):
                E = WINDOW_E[j]
                biasT = bias_pool.tile([P, 1024], F32, tag="biasT")
                nc.sync.dma_start(biasT, biasT_d[j])
                for G in (3, 2, 1, 0):
                    c_lo = max(0, 4 * G - E)
                    chunks = list(range(c_lo, 4 * G + 4))
                    probsT = attn_pool.tile([P, NB, 512], BF16, tag="pT")
                    for c in chunks:
                        d = c - 4 * G  # -12..3
                        lo = max(0, d) * P  # first causally-valid column
                        w = 512 - lo
                        ps = psA.tile([P, 512], F32, tag="ps")
                        nc.tensor.matmul(
                            ps[:, :w],
                            lhsT=kq_all[:, j, c * P : (c + 1) * P],
                            rhs=kq_all[:, 4 + j, G * 512 + lo : (G + 1) * 512],
                            start=True,
                            stop=True,
                        )
                        if d < 0:
                            nc.vector.tensor_tensor(
                                ps[:, :w], ps[:, :w], biasT[:, 0:512], OP.add
                            )
                        else:
                            # diagonal 128-block uses the causal-masked
                            # variant; the rest of the row uses the base
                            nc.vector.tensor_tensor(
                                ps[:, :P],
                                ps[:, :P],
                                biasT[:, 512 + d * P : 512 + (d + 1) * P],
                                OP.add,
                            )
                            if w > P:
                                nc.vector.tensor_tensor(
                                    ps[:, P:w],
                                    ps[:, P:w],
                                    biasT[:, lo + P : 512],
                                    OP.add,
                                )
                        nc.scalar.activation(
                            probsT[:, c, lo:],
                            ps[:, :w],
                            AF.Exp,
                            bias=negshT[:, j, d + 12 : d + 13],
                            scale=1.0,
                        )
                    # fold full-width chunks (d <= 0) into quads on DVE
                    full = [c for c in chunks if c <= 4 * G]
                    quads = fold_pool.tile(
                        [P, 4, 512], BF16, tag="fq"
                    )  # up to 4 quad sums per unit
                    rs_rhs = []  # APs for the full-width rowsum matmuls
                    for qi in range(0, len(full), 4):
                        grp = full[qi : qi + 4]
                        qslot = qi // 4
                        if len(grp) == 1:
                            rs_rhs.append(probsT[:, grp[0], :])
                            continue
                        if len(grp) >= 2:
                            t1 = fold_pool.tile([P, 512], BF16, tag="f1")
                            nc.vector.tensor_tensor(
                                t1,
                                probsT[:, grp[0], :],
                                probsT[:, grp[1], :],
                                OP.add,
                            )
                        if len(grp) == 2:
                            nc.vector.tensor_copy(quads[:, qslot, :], t1)
                        elif len(grp) == 3:
                            nc.vector.tensor_tensor(
                                quads[:, qslot, :], t1, probsT[:, grp[2], :], OP.add
                            )
                        else:
                            t2 = fold_pool.tile([P, 512], BF16, tag="f2")
                            nc.vector.tensor_tensor(
                                t2,
                                probsT[:, grp[2], :],
                                probsT[:, grp[3], :],
                                OP.add,
                            )
                            nc.vector.tensor_tensor(
                                quads[:, qslot, :], t1, t2, OP.add
                            )
                        rs_rhs.append(quads[:, qslot, :])
                    # rowsum: quads full-width, then diagonal partial ranges
                    rs_ps = psS.tile([1, 512], F32, tag="rs")
                    for qi, rhs_ap in enumerate(rs_rhs):
                        nc.tensor.matmul(
                            rs_ps,
                            lhsT=ones_bf,
                            rhs=rhs_ap,
                            start=(qi == 0),
                            stop=False,
                            skip_group_check=True,
                        )
                    for d in (1, 2, 3):
                        lo = d * P
                        nc.tensor.matmul(
                            rs_ps[:, lo:],
                            lhsT=ones_bf,
                            rhs=probsT[:, 4 * G + d, lo:],
                            start=False,
                            stop=(d == 3),
                            skip_group_check=True,
                        )
                    # PV: outT[hd, sq_group] accumulated over kept chunks,
                    # column-restricted on the diagonal
                    po = psO.tile([P, 512], F32, tag="po")
                    for i, c in enumerate(chunks):
                        lo = max(0, c - 4 * G) * P
                        nc.tensor.matmul(
                            po[:, lo:] if lo else po,
                            lhsT=v_all[:, c, j * HD : (j + 1) * HD],
                            rhs=probsT[:, c, lo:],
                            start=(i == 0),
                            stop=(i == len(chunks) - 1),
                            skip_group_check=(lo > 0),
                        )
                    # normalize: broadcast rowsum across partitions via K=1
                    # f32r matmul, reciprocal into SBUF, one DVE multiply
                    # (each op reads at most one PSUM operand)
                    rs_r = stats.tile([1, 512], F32R, tag="rs_r")
                    with nc.allow_low_precision(reason="f32r rounding only"):
                        nc.vector.tensor_copy(rs_r, rs_ps)
                    rb = psS.tile([P, 512], F32, tag="rb", bufs=1)
                    nc.tensor.matmul(
                        rb, lhsT=ones1_f, rhs=rs_r, start=True, stop=True
                    )
                    rb_rec = attn_pool.tile([P, 512], F32, tag="rbr")
                    nc.vector.reciprocal_approx_fast(rb_rec, rb)
                    out_sb = attn_pool.tile([P, 512], F32, tag="osb")
                    nc.vector.tensor_tensor(out_sb, po, rb_rec, OP.mult)
                    # + V-projection bias (sum of normalized probs == 1)
                    nc.scalar.activation(
                        out_sb,
                        out_sb,
                        AF.Identity,
                        bias=bvt_sb[:, j : j + 1],
                        scale=1.0,
                    )
                    nc.sync.dma_start(
                        out_d[j][:, G * 512 : (G + 1) * 512], out_sb
                    )

    nc.finalize()
    return nc


_NC_CACHE = None


def _get_nc():
    global _NC_CACHE
    if _NC_CACHE is None:
        _NC_CACHE = build()
    return _NC_CACHE


def _core_heads(hg):
    return [4 * jj + hg for jj in range(H_LOC)]


def _make_in_maps(x, W_kqv, b_kqv):
    x = np.asarray(x, np.float32)
    W = np.asarray(W_kqv, np.float32)
    b = np.asarray(b_kqv, np.float32)
    slopes = _alibi_slopes()
    in_maps = []
    for core in range(8):
        bi, hg = divmod(core, 4)
        heads = _core_heads(hg)
        xT = np.ascontiguousarray(x[bi].T).astype(ml_dtypes.bfloat16)
        wkq = np.concatenate(
            [W[:, h * HD : (h + 1) * HD] for h in heads]
            + [W[:, D + h * HD : D + (h + 1) * HD] for h in heads],
            axis=1,
        ).astype(ml_dtypes.bfloat16)
        wv = np.concatenate(
            [W[:, 2 * D + h * HD : 2 * D + (h + 1) * HD] for h in heads], axis=1
        ).astype(ml_dtypes.bfloat16)
        # bias columns: K s0..s3 then Q s0..s3; q-side prescaled by 1/sqrt(hd)
        bkq = np.stack(
            [b[h * HD : (h + 1) * HD] for h in heads]
            + [b[D + h * HD : D + (h + 1) * HD] * SCALE for h in heads],
            axis=1,
        ).astype(np.float32)
        bvt = np.stack(
            [b[2 * D + h * HD : 2 * D + (h + 1) * HD] for h in heads], axis=1
        ).astype(np.float32)  # [hd, H_LOC]
        # biasT[j, tl, 0:512] = base m_j*(tl-sqg); [j, tl, 512+128d:...]
        # = base diag block d plus -1e30 where tl > sql (causal mask)
        relT = (np.arange(P)[:, None] - np.arange(512)[None, :]).astype(np.float32)
        base = slopes[heads][:, None, None] * relT[None]  # [4, 128, 512]
        causal_blk = np.where(
            np.arange(P)[:, None] > np.arange(P)[None, :], -1e30, 0.0
        ).astype(np.float32)
        bias_t = np.zeros((H_LOC, P, 1024), np.float32)
        bias_t[:, :, 0:512] = base
        for dd in range(4):
            bias_t[:, :, 512 + dd * P : 512 + (dd + 1) * P] = (
                base[:, :, dd * P : (dd + 1) * P] + causal_blk[None]
            )
        # negshT[p, j, d+12] = m_j * 128 * d, d in [-12, 3]
        dvals = (np.arange(16) - 12).astype(np.float32) * P
        negsht = np.tile(
            (slopes[heads][:, None] * dvals[None, :])[None], (P, 1, 1)
        ).astype(np.float32)
        in_maps.append(
            dict(
                xT=xT, wKQ=wkq, wV=wv, bKQ=bkq, bVT=bvt,
                biasT=bias_t, negshT=negsht,
            )
        )
    return in_maps


def run(inputs, trace=False, **kw):
    nc = _get_nc()
    in_maps = _make_in_maps(inputs["x"], inputs["W_kqv"], inputs["b_kqv"])
    bkr = run_bass_kernel_spmd(nc, in_maps, core_ids=list(range(8)), trace=trace, **kw)
    B = 2
    out = np.empty((B, NUM_HEADS, S, HD), np.float32)
    for core in range(8):
        bi, hg = divmod(core, 4)
        heads = _core_heads(hg)
        o = np.asarray(bkr.results[core]["out"])  # [4, 128(hd), 2048(s)]
        for j in range(H_LOC):
            out[bi, heads[j]] = o[j].T
    return out, bkr


def kernel(x, W_kqv, b_kqv):
    out, _ = run({"x": x, "W_kqv": W_kqv, "b_kqv": b_kqv})
    return out


# revision 7
# speedup vs baseline: 1.2209x; 1.1281x over previous
"""ALiBi multi-head causal attention on 8 TRN2 NeuronCores.

Sharding: core = b*4 + hg (b in 0..1 batches, hg in 0..3).  Heads are
INTERLEAVED across cores: core (b, hg) owns heads [hg, 4+hg, 8+hg, 12+hg]
(slot j = head 4j+hg), so every core holds one head from each ALiBi-slope
quartile.  ALiBi decays exponentially per head; far-past key chunks are
skipped per-slot (window E chunks beyond the 4 diagonal chunks of each
512-query group; nearest dropped key sits 128E+1 back, worst dropped-key
bias <= -m_max*(128E+1) = {-32, -16, -12, -}).  Work per core:
19+22+32+40 = 113 chunk-units vs 160 causal, identical on every core.

Per-core kernel (all matmuls bf16, f32 accumulation):
  - DMAs are ordered critical-first (wkq m=0 k-quarters + x block 0 ahead
    of the bulk) so the first matmul issues right after the ~8us engine
    preamble instead of behind the full 10 MB input flood.
  - The KQV projection (4 blocks of 512 sequence positions) is INTERLEAVED
    with attention: after projection block G, the four attention units
    (slot j, q-group G) are emitted, so projection matmuls fill the PE
    while attention's DVE/ACT chains drain, and vice versa.
  - Attention runs in TRANSPOSED score space scoreT[t, sq] (k stationary,
    q-group moving), so PV consumes probsT directly with no transposes.
  - ALiBi bias, slots 1-3 (max slope 2^-2.5): RANK-1 path — the bias
    m*(t-sq) splits into a per-partition part m*(t - sq_ref) folded into
    the EXP's bias vector (sq_ref = group center keeps exponents in ~+-45,
    no f32/bf16 overflow for m <= 0.177) and a per-column factor
    exp(m*(sq-sq_ref)) that CANCELS in the softmax normalization.  Only
    the causal mask of the 4 diagonal 128-blocks needs a [128,128] DVE
    add.  Slot 0 (slopes up to 0.7) keeps the full 2D bias add (f32 range
    cannot span exp(m*512)): base bias over [lo:512] plus a causal-masked
    diagonal-block variant, both from a compressed [128,1024] table.
  - rowsum over t: full-width chunks fold in bf16 quad-trees on the Vector
    engine, then one M=1 ones-matmul per quad; diagonal chunks d=1..3 get
    column-restricted ([128d:512]) rowsum matmuls.  No memsets anywhere;
    PV accumulation is column-restricted the same way.
  - normalize: rowsum copied to SBUF, broadcast across partitions with a
    stride-0-source DMA, reciprocal_approx_fast, one DVE multiply.  The
    V-projection bias is pre-added into v_all during the projection
    epilogue (sum of normalized probs == 1), so the tail has no ACT op.
  - output written as outT [slot, hd, s]; host transposes/reorders back.
"""

import sys

if "/opt/trn_rl_repo" not in sys.path:
    sys.path.insert(0, "/opt/trn_rl_repo")

import numpy as np
import ml_dtypes

import concourse.bass as bass
import concourse.mybir as mybir
from concourse import bacc
from concourse.tile import TileContext
from concourse.bass_utils import run_bass_kernel_spmd

P = 128
S = 2048
D = 2048
HD = 128
NB = S // P            # 16 seq blocks
H_LOC = 4              # heads per core
NUM_HEADS = 16
SCALE = 1.0 / np.sqrt(HD)

# chunks kept beyond the diagonal 4, per head-slot (slot j = head 4j+hg).
# Verified truncation rel-err 1.5e-6 vs full causal on reference inputs.
WINDOW_E = (1, 2, 6, 16)
# slots whose max slope allows the rank-1 exp-bias path (m*256 < 60)
RANK1_MIN_SLOT = 1

F32 = mybir.dt.float32
F32R = mybir.dt.float32r
BF16 = mybir.dt.bfloat16
AF = mybir.ActivationFunctionType
OP = mybir.AluOpType


def _alibi_slopes(num_heads=NUM_HEADS):
    base = (2.0 ** 8) ** (1.0 / num_heads)
    return np.asarray([1.0 / base ** (i + 1) for i in range(num_heads)], np.float32)


def build():
    nc = bacc.Bacc("TRN2", target_bir_lowering=False)

    xT_d = nc.declare_dram_parameter("xT", [D, S], BF16, isOutput=False)
    wKQ_d = nc.declare_dram_parameter("wKQ", [D, 8 * P], BF16, isOutput=False)
    wV_d = nc.declare_dram_parameter("wV", [D, H_LOC * HD], BF16, isOutput=False)
    bKQ_d = nc.declare_dram_parameter("bKQ", [P, 8], F32, isOutput=False)
    # V bias pre-broadcast to all partitions: bvtb[p, j*128+d] = b_v[head_j, d]
    bVTB_d = nc.declare_dram_parameter("bVTB", [P, H_LOC * HD], F32, isOutput=False)
    # slot-0 2D bias table, compressed: [:, 0:512] = base m0*(tl-sqg);
    # [:, 512+128d : 512+128(d+1)] = base diag block d + causal -1e30 mask
    biasT0_d = nc.declare_dram_parameter("biasT0", [P, 1024], F32, isOutput=False)
    # causal mask for one diagonal 128-block: -1e30 where tl > sql
    maskT_d = nc.declare_dram_parameter("maskT", [P, P], F32, isOutput=False)
    # EXP bias: slot 0: m0*128*d (tiled);  slots 1-3: m_j*(tl + 128d - 255)
    negshT_d = nc.declare_dram_parameter("negshT", [P, H_LOC, 16], F32, isOutput=False)
    # out in transposed-per-slot layout [slot, hd, s]; host transposes back
    out_d = nc.declare_dram_parameter("out", [H_LOC, HD, S], F32, isOutput=True)

    xT_t = xT_d.rearrange("(ko p) s -> p ko s", p=P)     # [128, 16, 2048]
    wKQ_t = wKQ_d.rearrange("(ko p) n -> p ko n", p=P)   # [128, 16, 1024]
    wV_t = wV_d.rearrange("(ko p) n -> p ko n", p=P)     # [128, 16, 512]

    with TileContext(nc) as tc:
        with (
            tc.tile_pool(name="const", bufs=1) as const,
            tc.tile_pool(name="resid", bufs=1) as resid,
            tc.tile_pool(name="stats", bufs=2) as stats,
            tc.tile_pool(name="psA", bufs=3, space="PSUM") as psA,
            tc.tile_pool(name="psO", bufs=2, space="PSUM") as psO,
            tc.tile_pool(name="psS", bufs=2, space="PSUM") as psS,
            tc.tile_pool(name="wpool", bufs=1) as wpool,
            tc.tile_pool(name="xpool", bufs=2) as xpool,
            tc.tile_pool(name="attn", bufs=2) as attn_pool,
            tc.tile_pool(name="fold", bufs=2) as fold_pool,
        ):
            # ---- tiles; DMA issue order is the startup-critical path ----
            wkq_sb = wpool.tile([P, 16, 8 * P], BF16)
            wv_sb = wpool.tile([P, 16, H_LOC * HD], BF16)
            bkq_sb = const.tile([P, 8], F32)
            bvtb_sb = const.tile([P, H_LOC * HD], F32)
            biasT0 = const.tile([P, 1024], F32)
            maskT = const.tile([P, P], F32)
            negshT = const.tile([P, H_LOC, 16], F32)

            # sync queue: wkq m=0 in k-quarters first, then the rest
            for kk in range(4):
                nc.sync.dma_start(
                    wkq_sb[:, 4 * kk : 4 * kk + 4, 0:P],
                    wKQ_t[:, 4 * kk : 4 * kk + 4, 0:P],
                )
            nc.sync.dma_start(bkq_sb, bKQ_d[:])
            for m in range(1, 8):
                nc.sync.dma_start(
                    wkq_sb[:, :, m * P : (m + 1) * P],
                    wKQ_t[:, :, m * P : (m + 1) * P],
                )
            nc.sync.dma_start(wv_sb, wV_t)
            nc.sync.dma_start(bvtb_sb, bVTB_d[:])
            nc.sync.dma_start(biasT0, biasT0_d[:])
            nc.sync.dma_start(maskT, maskT_d[:])
            nc.sync.dma_start(negshT, negshT_d[:])

            ones_bf = const.tile([P, 1], BF16)  # rowsum column
            nc.gpsimd.memset(ones_bf, 1.0)
            ones1_raw = const.tile([1, P], F32)
            nc.gpsimd.memset(ones1_raw, 1.0)
            ones1_f = const.tile([1, P], F32R)  # partition-broadcast row
            with nc.allow_low_precision(reason="constant ones cast to f32r"):
                nc.vector.tensor_copy(ones1_f, ones1_raw)

            # ---- residents ----
            kq_all = resid.tile([P, 8, S], BF16)       # [hd, (K s0..3 | Q s0..3), s]
            v_all = resid.tile([P, NB, H_LOC * HD], BF16)  # [si, so, j*128+d]

            def proj_block(nb):
                xc = xpool.tile([P, 16, 512], BF16, tag="xc")
                for kk in range(4):
                    nc.scalar.dma_start(
                        xc[:, 4 * kk : 4 * kk + 4, :],
                        xT_t[:, 4 * kk : 4 * kk + 4, nb * 512 : (nb + 1) * 512],
                    )
                for m in range(8):
                    ps = psA.tile([P, 512], F32, tag="ps")
                    for k in range(16):
                        nc.tensor.matmul(
                            ps,
                            lhsT=wkq_sb[:, k, m * P : (m + 1) * P],
                            rhs=xc[:, k, :],
                            start=(k == 0),
                            stop=(k == 15),
                        )
                    # kqT = psum * scale + bias (scale folds 1/sqrt(hd) into q)
                    nc.scalar.activation(
                        kq_all[:, m, nb * 512 : (nb + 1) * 512],
                        ps,
                        AF.Identity,
                        bias=bkq_sb[:, m : m + 1],
                        scale=float(SCALE) if m >= 4 else 1.0,
                    )
                for sub in range(4):
                    s_idx = nb * 4 + sub
                    psv = psA.tile([P, 512], F32, tag="ps")
                    for k in range(16):
                        nc.tensor.matmul(
                            psv,
                            lhsT=xc[:, k, sub * P : (sub + 1) * P],
                            rhs=wv_sb[:, k, :],
                            start=(k == 0),
                            stop=(k == 15),
                        )
                    # v = psum + b_v (pre-added so the attention tail is ACT-free)
                    nc.vector.tensor_tensor(
                        v_all[:, s_idx, :], psv, bvtb_sb, OP.add
                    )

            def attn_unit(j, G):
                E = WINDOW_E[j]
                rank1 = j >= RANK1_MIN_SLOT
                c_lo = max(0, 4 * G - E)
                chunks = list(range(c_lo, 4 * G + 4))
                probsT = attn_pool.tile([P, NB, 512], BF16, tag="pT")
                for c in chunks:
                    d = c - 4 * G  # -12..3
                    lo = max(0, d) * P  # first causally-valid column
                    w = 512 - lo
                    ps = psA.tile([P, 512], F32, tag="ps")
                    nc.tensor.matmul(
                        ps[:, :w],
                        lhsT=kq_all[:, j, c * P : (c + 1) * P],
                        rhs=kq_all[:, 4 + j, G * 512 + lo : (G + 1) * 512],
                        start=True,
                        stop=True,
                    )
                    if rank1:
                        # bias handled by EXP's per-partition vector; only the
                        # diagonal 128-block needs the causal mask added
                        if d >= 0:
                            nc.vector.tensor_tensor(
                                ps[:, :P], ps[:, :P], maskT, OP.add
                            )
                    elif d < 0:
                        nc.vector.tensor_tensor(
                            ps[:, :w], ps[:, :w], biasT0[:, 0:512], OP.add
                        )
                    else:
                        nc.vector.tensor_tensor(
                            ps[:, :P],
                            ps[:, :P],
                            biasT0[:, 512 + d * P : 512 + (d + 1) * P],
                            OP.add,
                        )
                        if w > P:
                            nc.vector.tensor_tensor(
                                ps[:, P:w], ps[:, P:w], biasT0[:, lo + P : 512],
                                OP.add,
                            )
                    nc.scalar.activation(
                        probsT[:, c, lo:],
                        ps[:, :w],
                        AF.Exp,
                        bias=negshT[:, j, d + 12 : d + 13],
                        scale=1.0,
                    )
                # fold full-width chunks (d <= 0) into quads on DVE
                full = [c for c in chunks if c <= 4 * G]
                quads = fold_pool.tile([P, 4, 512], BF16, tag="fq")
                rs_rhs = []
                for qi in range(0, len(full), 4):
                    grp = full[qi : qi + 4]
                    qslot = qi // 4
                    if len(grp) == 1:
                        rs_rhs.append(probsT[:, grp[0], :])
                        continue
                    t1 = fold_pool.tile([P, 512], BF16, tag="f1")
                    nc.vector.tensor_tensor(
                        t1, probsT[:, grp[0], :], probsT[:, grp[1], :], OP.add
                    )
                    if len(grp) == 2:
                        nc.vector.tensor_copy(quads[:, qslot, :], t1)
                    elif len(grp) == 3:
                        nc.vector.tensor_tensor(
                            quads[:, qslot, :], t1, probsT[:, grp[2], :], OP.add
                        )
                    else:
                        t2 = fold_pool.tile([P, 512], BF16, tag="f2")
                        nc.vector.tensor_tensor(
                            t2, probsT[:, grp[2], :], probsT[:, grp[3], :], OP.add
                        )
                        nc.vector.tensor_tensor(quads[:, qslot, :], t1, t2, OP.add)
                    rs_rhs.append(quads[:, qslot, :])
                # rowsum: quads full-width, then diagonal partial ranges
                rs_ps = psS.tile([1, 512], F32, tag="rs")
                for qi, rhs_ap in enumerate(rs_rhs):
                    nc.tensor.matmul(
                        rs_ps,
                        lhsT=ones_bf,
                        rhs=rhs_ap,
                        start=(qi == 0),
                        stop=False,
                        skip_group_check=True,
                    )
                for d in (1, 2, 3):
                    lo = d * P
                    nc.tensor.matmul(
                        rs_ps[:, lo:],
                        lhsT=ones_bf,
                        rhs=probsT[:, 4 * G + d, lo:],
                        start=False,
                        stop=(d == 3),
                        skip_group_check=True,
                    )
                # PV: outT[hd, sq_group] accumulated, column-restricted
                po = psO.tile([P, 512], F32, tag="po")
                for i, c in enumerate(chunks):
                    lo = max(0, c - 4 * G) * P
                    nc.tensor.matmul(
                        po[:, lo:] if lo else po,
                        lhsT=v_all[:, c, j * HD : (j + 1) * HD],
                        rhs=probsT[:, c, lo:],
                        start=(i == 0),
                        stop=(i == len(chunks) - 1),
                        skip_group_check=(lo > 0),
                    )
                # normalize: rowsum broadcast across partitions via a K=1
                # f32r matmul, reciprocal into SBUF, one DVE multiply
                rs_r = stats.tile([1, 512], F32R, tag="rs_r")
                with nc.allow_low_precision(reason="f32r rounding only"):
                    nc.vector.tensor_copy(rs_r, rs_ps)
                rb = psS.tile([P, 512], F32, tag="rb", bufs=1)
                nc.tensor.matmul(
                    rb, lhsT=ones1_f, rhs=rs_r, start=True, stop=True
                )
                rb_rec = attn_pool.tile([P, 512], F32, tag="rbr")
                nc.vector.reciprocal_approx_fast(rb_rec, rb)
                out_sb = attn_pool.tile([P, 512], F32, tag="osb")
                nc.vector.tensor_tensor(out_sb, po, rb_rec, OP.mult)
                nc.sync.dma_start(out_d[j][:, G * 512 : (G + 1) * 512], out_sb)

            # ---- interleaved schedule: projection block G, then the four
            # attention units of q-group G (their K/V/Q blocks are ready) ----
            for G in range(4):
                proj_block(G)
                for j in range(H_LOC):
                    attn_unit(j, G)

    nc.finalize()
    return nc


_NC_CACHE = None


def _get_nc():
    global _NC_CACHE
    if _NC_CACHE is None:
        _NC_CACHE = build()
    return _NC_CACHE


def _core_heads(hg):
    return [4 * jj + hg for jj in range(H_LOC)]


def _make_in_maps(x, W_kqv, b_kqv):
    x = np.asarray(x, np.float32)
    W = np.asarray(W_kqv, np.float32)
    b = np.asarray(b_kqv, np.float32)
    slopes = _alibi_slopes()
    in_maps = []
    for core in range(8):
        bi, hg = divmod(core, 4)
        heads = _core_heads(hg)
        m_h = slopes[heads]  # per-slot slopes
        xT = np.ascontiguousarray(x[bi].T).astype(ml_dtypes.bfloat16)
        wkq = np.concatenate(
            [W[:, h * HD : (h + 1) * HD] for h in heads]
            + [W[:, D + h * HD : D + (h + 1) * HD] for h in heads],
            axis=1,
        ).astype(ml_dtypes.bfloat16)
        wv = np.concatenate(
            [W[:, 2 * D + h * HD : 2 * D + (h + 1) * HD] for h in heads], axis=1
        ).astype(ml_dtypes.bfloat16)
        # bias columns: K s0..s3 then Q s0..s3; q-side prescaled by 1/sqrt(hd)
        bkq = np.stack(
            [b[h * HD : (h + 1) * HD] for h in heads]
            + [b[D + h * HD : D + (h + 1) * HD] * SCALE for h in heads],
            axis=1,
        ).astype(np.float32)
        # V bias pre-broadcast to all 128 partitions
        bvtb = np.tile(
            np.concatenate([b[2 * D + h * HD : 2 * D + (h + 1) * HD] for h in heads])[
                None, :
            ],
            (P, 1),
        ).astype(np.float32)
        # slot-0 compressed 2D bias table
        relT = (np.arange(P)[:, None] - np.arange(512)[None, :]).astype(np.float32)
        base0 = m_h[0] * relT  # [128, 512]
        causal_blk = np.where(
            np.arange(P)[:, None] > np.arange(P)[None, :], -1e30, 0.0
        ).astype(np.float32)
        bias_t0 = np.zeros((P, 1024), np.float32)
        bias_t0[:, 0:512] = base0
        for dd in range(4):
            bias_t0[:, 512 + dd * P : 512 + (dd + 1) * P] = (
                base0[:, dd * P : (dd + 1) * P] + causal_blk
            )
        # EXP bias table [p, j, d+12]:
        #   slot 0 (2D path):  m0 * 128 * d            (partition-constant)
        #   slots 1-3 (rank1): m_j * (tl + 128d - 255) (per-partition)
        dvals = (np.arange(16) - 12).astype(np.float32) * P  # 128*d
        negsht = np.empty((P, H_LOC, 16), np.float32)
        negsht[:, 0, :] = m_h[0] * dvals[None, :]
        tl = np.arange(P, dtype=np.float32)
        for jj in range(1, H_LOC):
            negsht[:, jj, :] = m_h[jj] * (tl[:, None] + dvals[None, :] - 255.0)
        in_maps.append(
            dict(
                xT=xT, wKQ=wkq, wV=wv, bKQ=bkq, bVTB=bvtb,
                biasT0=bias_t0, maskT=causal_blk, negshT=negsht,
            )
        )
    return in_maps


def run(inputs, trace=False, **kw):
    nc = _get_nc()
    in_maps = _make_in_maps(inputs["x"], inputs["W_kqv"], inputs["b_kqv"])
    bkr = run_bass_kernel_spmd(nc, in_maps, core_ids=list(range(8)), trace=trace, **kw)
    B = 2
    out = np.empty((B, NUM_HEADS, S, HD), np.float32)
    for core in range(8):
        bi, hg = divmod(core, 4)
        heads = _core_heads(hg)
        o = np.asarray(bkr.results[core]["out"])  # [4, 128(hd), 2048(s)]
        for j in range(H_LOC):
            out[bi, heads[j]] = o[j].T
    return out, bkr


def kernel(x, W_kqv, b_kqv):
    out, _ = run({"x": x, "W_kqv": W_kqv, "b_kqv": b_kqv})
    return out
